# revision 12
# baseline (speedup 1.0000x reference)
"""BetweennessRoPE Trainium2 kernel — fixed-table interpolated RoPE.

Math derivation (from the reference):
  score = relu(1 - (path-direct)/max(direct,1e-6)) lies in [0,1] by the
  triangle inequality, so between = score/2046 in [0, 4.887e-4] and
  pos_adj = -0.05 + between*0.1 spans only 4.887e-5.  Hence for s>=1:
  lo = s-1, hi = s, frac = 0.95 + between*0.1.  Freezing frac at the
  midpoint makes the interpolated cos/sin tables constants:
      C[s,k] = (1-fr)*cos((s-1)b_k) + fr*cos(s b_k)   (s>=1), C[0,k]=1
      Sn[s,k] likewise from sin, Sn[0,k]=0
  and the whole module collapses to plain RoPE with those tables:
      out[..., 2k]   = x[2k]*C - x[2k+1]*Sn
      out[..., 2k+1] = x[2k+1]*C + x[2k]*Sn
  The dropped score term perturbs frac by <=2.44e-5 -> output error
  ~1e-4 of scale; fp16 I/O + compute adds ~1e-3.  Gate is 2e-2.

Implementation: rotate-half form with duplicated/signed tables so the
pair swap is a negative-step access pattern (stays in DVE 2x mode):
      m1 = x * CD          CD[2k]=CD[2k+1]=C[k]
      m2 = swap(x) * SD    SD[2k]=-Sn[k], SD[2k+1]=+Sn[k]
      out = m1 + m2
Sharding: core c owns positions s in [256c, 256(c+1)) as 2 partition
blocks of 128; free dim packs (sb, b, h, d) = 16384 cols per core in
16 contiguous DRAM chunks of 1024, processed in variable-width stages
(1024-col first stages so DVE starts once 256 KiB lands; 1024-col last
stages so the drain is short).  The combine runs on PE as +identity
matmul pairs (PSUM f32) cast back to f16 by Act for the bulk, and as
DVE adds interleaved with the final multiplies for the tail stages.
All fp16; DVE/DMA-bound (~18.4 us of 2x multiplies, ~8.4 MiB HBM).
"""

import numpy as np

B, S, H, D = 4, 2048, 16, 128
NCORES = 8
NSB = 2
K2 = D // 2
WT = NSB * B * H * D     # 16384 cols per core
CHW = 1024               # DRAM chunk width
NCH = WT // CHW          # 16 chunks
# stage widths in chunks; stages 0-4 cover sb0, 5-9 cover sb1
STAGES = (1, 1, 2, 2, 2, 2, 2, 2, 1, 1)
PE_STAGES = (0, 1, 2, 3, 4, 5, 6)    # combined on PE+Act; rest DVE adds
FR = 0.95 + 0.5 / 2046.0 * 0.1

_cache = {}


def _make_tables():
    """Duplicated cos / signed sin tables [S, 128] f16."""
    k = np.arange(K2, dtype=np.float64)
    base = 1.0 / (10000.0 ** (2.0 * k / D))
    ang = np.arange(S, dtype=np.float64)[:, None] * base[None, :]
    fcos, fsin = np.cos(ang), np.sin(ang)
    lo = np.maximum(np.arange(S) - 1, 0)
    C = (1.0 - FR) * fcos[lo] + FR * fcos
    Sn = (1.0 - FR) * fsin[lo] + FR * fsin
    C[0, :] = 1.0
    Sn[0, :] = 0.0
    CD = np.repeat(C, 2, axis=1)
    SD = np.empty((S, D), np.float64)
    SD[:, 0::2] = -Sn
    SD[:, 1::2] = Sn
    return CD.astype(np.float16), SD.astype(np.float16)


def _build_nc():
    import concourse.bacc as bacc
    import concourse.mybir as mybir
    from concourse.tile import TileContext

    f16 = mybir.dt.float16
    f32 = mybir.dt.float32

    nc = bacc.Bacc()
    X = nc.dram_tensor("X", [NCH, 128, CHW], f16, kind="ExternalInput")
    TW = NSB * 2 * D + 128          # [cd0|sd0|cd1|sd1|I]
    TAB = nc.dram_tensor("TAB", [128, TW], f16, kind="ExternalInput")
    OUT = nc.dram_tensor("OUT", [NCH, 128, CHW], f16, kind="ExternalOutput")

    coffs = [0]
    for w in STAGES:
        coffs.append(coffs[-1] + w)
    assert coffs[-1] == NCH

    def dma_stage(T, st, sbuf_ap, to_sbuf):
        c0, c1 = coffs[st], coffs[st + 1]
        if c1 - c0 == 1:
            dram = T[c0]
            sb_view = sbuf_ap
        else:
            dram = T[c0:c1].rearrange("c p w -> p c w")
            sb_view = sbuf_ap.rearrange("p (c w) -> p c w", c=c1 - c0)
        if to_sbuf:
            nc.sync.dma_start(sb_view, dram)
        else:
            nc.sync.dma_start(dram, sb_view)

    with TileContext(nc) as tc:
        with (
            tc.tile_pool(name="tab", bufs=1) as tabp,
            tc.tile_pool(name="xin", bufs=1) as xinp,
            tc.tile_pool(name="prod", bufs=1) as prodp,
            tc.tile_pool(name="out", bufs=4) as outp,
            tc.tile_pool(name="odve", bufs=3) as odvep,
            tc.tile_pool(name="ps", bufs=2, space="PSUM") as psp,
        ):
            tab = tabp.tile([128, TW], f16, tag="tab", name="tab")
            idt = tab[:, NSB * 2 * D:NSB * 2 * D + 128]

            xts = []
            for st, wc in enumerate(STAGES):
                w = wc * CHW
                x = xinp.tile([128, w], f16, tag=f"x{st}", name=f"x{st}")
                dma_stage(X, st, x[:, :], True)
                xts.append(x)
                if st == 0:
                    # tiny table load in parallel on the Act DGE
                    nc.scalar.dma_start(tab[:, :], TAB[:, :])

            m1s, m2s = [], []
            dve_outs = []
            for st, wc in enumerate(STAGES):
                w = wc * CHW
                sb = 0 if coffs[st] < NCH // 2 else 1
                nj = w // D
                x = xts[st]
                cb = tab[:, sb * 2 * D:sb * 2 * D + D].unsqueeze(
                    1).broadcast_to([128, nj, D])
                sdb = (tab[:, sb * 2 * D + D:sb * 2 * D + 2 * D]
                       .rearrange("p (k two) -> p k two", two=2)
                       .unsqueeze(1).broadcast_to([128, nj, K2, 2]))
                xv = x[:, :].rearrange("p (j d) -> p j d", d=D)
                xsw = x[:, :].rearrange(
                    "p (j k two) -> p j k two", two=2, k=K2)[:, :, :, ::-1]
                m1 = prodp.tile([128, w], f16, tag=f"m1_{st}",
                                name=f"m1_{st}")
                m2 = prodp.tile([128, w], f16, tag=f"m2_{st}",
                                name=f"m2_{st}")
                m1v = m1[:, :].rearrange("p (j d) -> p j d", d=D)
                m2v = m2[:, :].rearrange(
                    "p (j k two) -> p j k two", two=2, k=K2)
                nc.vector.tensor_mul(m1v, xv, cb)
                nc.vector.tensor_mul(m2v, xsw, sdb)
                m1s.append(m1)
                m2s.append(m2)
                if st not in PE_STAGES:
                    # DVE combine, interleaved right after this stage's muls;
                    # the out-DMA is deferred so the in-order sync sequencer
                    # issues the PE stages' (earlier-ready) DMAs first
                    o = odvep.tile([128, w], f16, tag="od", name=f"o{st}")
                    nc.vector.tensor_add(o[:, :], m1[:, :], m2[:, :])
                    dve_outs.append((st, o))

            for st in PE_STAGES:
                w = STAGES[st] * CHW
                o = outp.tile([128, w], f16, tag="o", name=f"o{st}")
                ps = psp.tile([128, w], f32, tag="ps", name=f"ps{st}")
                for q in range(w // 512):
                    qs = slice(512 * q, 512 * (q + 1))
                    nc.tensor.matmul(ps[:, qs], idt, m1s[st][:, qs],
                                     start=True, stop=False)
                    nc.tensor.matmul(ps[:, qs], idt, m2s[st][:, qs],
                                     start=False, stop=True)
                nc.scalar.copy(o[:, :], ps[:, :])
                dma_stage(OUT, st, o[:, :], False)
            for st, o in dve_outs:
                dma_stage(OUT, st, o[:, :], False)
    nc.compile()
    return nc


def _get_built():
    if "nc" not in _cache:
        _cache["nc"] = _build_nc()
    return _cache["nc"]


def kernel(x, W, b):
    from concourse.bass_utils import run_bass_kernel_spmd

    assert x.shape == (B, S, H, D)
    # s = 256*c + 128*sb + p; per-core cols = (sb, b, h, d) in 1024-chunks
    x6 = np.asarray(x, dtype=np.float32).reshape(
        B, NCORES, NSB, 128, H, D).astype(np.float16)
    xs = np.ascontiguousarray(x6.transpose(1, 2, 0, 3, 4, 5)).reshape(
        NCORES, NSB * B, 128, H * D)
    # [c, (sb b), p, (h d)] -> chunks of 1024: split (h d)=2048 in half
    xs = xs.reshape(NCORES, NSB * B, 128, 2, CHW).transpose(0, 1, 3, 2, 4)
    xs = np.ascontiguousarray(xs).reshape(NCORES, NCH, 128, CHW)

    if "tabs" not in _cache:
        CDf, SDf = _make_tables()      # [S, 128]
        cc = CDf.reshape(NCORES, NSB, 128, D)
        ss = SDf.reshape(NCORES, NSB, 128, D)
        tabs = np.empty((NCORES, 128, NSB * 2 * D + 128), np.float16)
        for sb in range(NSB):
            tabs[:, :, sb * 2 * D:sb * 2 * D + D] = cc[:, sb]
            tabs[:, :, sb * 2 * D + D:sb * 2 * D + 2 * D] = ss[:, sb]
        tabs[:, :, NSB * 2 * D:] = np.eye(128, dtype=np.float16)[None]
        _cache["tabs"] = np.ascontiguousarray(tabs)
    tabs = _cache["tabs"]

    nc = _get_built()
    in_maps = []
    for c in range(NCORES):
        in_maps.append({"X": xs[c], "TAB": tabs[c]})
    res = run_bass_kernel_spmd(nc, in_maps, core_ids=list(range(NCORES)))
    if res.exec_time_ns is not None:
        print(f"HW exec time: {res.exec_time_ns} ns")

    outs = np.stack([res.results[c]["OUT"] for c in range(NCORES)])
    # [c, ch=(sb b half), p, 1024] -> [b, (c sb p), h, d]
    full = outs.reshape(NCORES, NSB * B, 2, 128, CHW).transpose(0, 1, 3, 2, 4)
    full = full.reshape(NCORES, NSB, B, 128, H, D).transpose(2, 0, 1, 3, 4, 5)
    return np.ascontiguousarray(full.reshape(B, S, H, D).astype(np.float32))
